# revision 41
# baseline (speedup 1.0000x reference)
"""Chamfer-style point loss (nn_PointLoss) on 8 Trainium2 NeuronCores.

Math (reference): reflect points across plane n.x+d=0; half1 = reflected
points (valid where s=p.n+d < 0, mask m1), half2 = original points (mask
m2 = ~m1). D[i,j] = ||half1[i]-half2[j]||^2. Output scalar =
50*(sum_j min_i(D) m2_j / c2 + sum_i min_j(D) m1_i / c1).

Formulation: each core computes TWO blocks of the negated-penalized
distance matrix -(D + P*rowpen + P*colpen):
  block A: its own 512 rows x all 4096 cols  -> row maxes are fully
           local -> masked sum s2_c (sentinel filter, no mask tensors)
  block B: all 4096 rows x its own 512 cols  -> col maxes are fully
           local -> masked sum s1_c
so each core emits one partial scalar out_c = -50*(s1_c/c2 + s2_c/c1)
and the only cross-core step is an 8-way scalar sum (host gather by
default, optional 4-byte on-device AllReduce via USE_AR). This removes
the 17KB AllReduce(max) + mesh-barrier of the column-sharded-only
formulation (~100us+ of collective latency/skew) at the cost of 2x
matmuls, which are cheap (PE is nowhere near the bottleneck).

Device pipeline: per-core input = one [128, 101] tensor: the full point
set ROLLED so its own 512 points come first (point = 32*p + slot),
plus a ones column and the plane normal pre-broadcast host-side. Prep
builds negated reflected coords (A side, linear identity |a|^2 =
|p|^2 + (4d/nn)s), -2p (B side), and rr/cc rank-1 rows with penalties
folded in; hi/lo bf16 splits (casts on ACT, the fast engine for them)
form two flat [128, 13*32] composites. A chunked DRAM round trip (64B-
run scatter writes, contiguous reads, sync/scalar hwdge queues) yields
K-major images [13, 128, 32]; K=13 drops the negligible aL*bL term and
needs no zero rows. 64 K=13 bf16 matmuls stream into fp32 PSUM; block-A
tiles get a single DVE max-reduce straight from PSUM; block-B tiles are
converted to fp16 by ACT and max-accumulated on DVE (2x mode) into two
CM halves; each half's cross-partition column-max is done by four PE
transposes (host-supplied fp16 identity) + one batched DVE reduce, so
the first half overlaps the loop and no gpsimd partition_all_reduce is
needed. Sentinel filtering (penalized entries < -8000) replaces all
mask tensors/reshuffles; c1/c2 are computed identically on every core.
Staging chunks are separate tiles so the first matmuls depend only on
chunk-0 DMAs (Tile coarsens dependencies per tile/semaphore-count); the
first chunk is half-size (16 i-slots) to cut its scatter-transfer
latency. Eight throwaway matmuls on the ready composite precede the
loop to nudge the PE clock.

Measured: 61.7-62.9us across 10 HW runs (vs 133us baseline), rel err 1.4e-05. The
remaining time is ~7us fixed NEFF startup, ~15us prep+staging (gated by
scatter-DMA transfer + prep chain depth), ~30us loop (DVE saturated at
~930ns per tile pair: 1x tensor_reduce from PSUM + 2x fp16 max; no
other engine can reduce or max), ~5us epilogue, ~10us fixed teardown.
"""

import os
import sys

import numpy as np

for _p in ("/opt/trn_rl_repo", "/root/.axon_site/_ro/trn_rl_repo"):
    if os.path.isdir(_p) and _p not in sys.path:
        sys.path.insert(0, _p)

import concourse.bacc as bacc
import concourse.bass_isa as bass_isa
import concourse.tile as tile
from concourse import mybir
from concourse.bass_utils import run_bass_kernel_spmd

FP = mybir.dt.float32
BF = mybir.dt.bfloat16
HF = mybir.dt.float16
AX = mybir.AxisListType
OP = mybir.AluOpType

N = 4096
NCORES = 8
G = 32              # image i-dim chunks of 32 points each; point = 32*i + s
PEN = float(2**14)  # row/col penalty, keeps penalized -(F) finite in fp16
SENT = -8000.0      # sentinel threshold: valid maxes are > -1000
CMINIT = -60000.0
NWARM = 16
USE_AR = False      # False: host sums 8 partial scalars; True: device AllReduce


def _emit(tc, out_ap, pa_ap, idn_ap):
    nc = tc.nc

    psf = tc.alloc_tile_pool(name="psf", bufs=3, space="PSUM")
    pss = tc.alloc_tile_pool(name="pss", bufs=1, space="PSUM")
    ptp = tc.alloc_tile_pool(name="ptp", bufs=1, space="PSUM")
    per = tc.alloc_tile_pool(name="per", bufs=1)
    fsp = tc.alloc_tile_pool(name="fsp", bufs=3)
    drm = tc.alloc_tile_pool(name="drm", bufs=1, space="DRAM")

    def _t(shape, name, dt=FP):
        return per.tile(shape, dt, name=name)

    # ---- single input tensor: [p, 0:96]=coords (c*32+s), 96=ones,
    # 97:101 = plane norm pre-broadcast host-side
    PA = _t([128, 101], "PA")
    nc.sync.dma_start(PA[:, 0:64], pa_ap[:, 0:64])
    nc.scalar.dma_start(PA[:, 64:101], pa_ap[:, 64:101])
    IDN = _t([128, 128], "IDN", HF)
    nc.scalar.dma_start(IDN[:], idn_ap[:])
    NB = PA[:, 97:101]
    ones_c = PA[:, 96:97]

    # ---- composite consts (zeros rows are skipped entirely via K=13)
    CALL_A = _t([128, 416], "CALL_A", BF)  # [p, k*32 + s], k<13
    nc.gpsimd.memset(CALL_A[:, 352:416], -1.0)  # k=11,12 -> -1
    CALL_B = _t([128, 416], "CALL_B", BF)
    nc.vector.memset(CALL_B[:, 288:352], 1.0)  # k=9,10 -> +1
    CM1 = _t([128, 512], "CM1", HF)
    nc.vector.memset(CM1[:], CMINIT)
    ones128 = _t([128, 1], "ones128")
    nc.gpsimd.memset(ones128[:], 1.0)

    # B side first: bn = -2p depends only on the input
    BN = _t([128, 96], "BN")
    nc.vector.tensor_scalar_mul(BN[:], PA[:, 0:96], -2.0)
    LOB = _t([128, 96], "LOB")
    nc.scalar.copy(CALL_B[:, 0:96], BN[:])
    nc.scalar.copy(CALL_B[:, 192:288], BN[:])
    nc.vector.tensor_tensor(LOB[:], BN[:], CALL_B[:, 0:96], op=OP.subtract)
    nc.scalar.copy(CALL_B[:, 96:192], LOB[:])

    # |p|^2 also depends only on the input
    pp = _t([128, 32], "pp")
    q1 = _t([128, 32], "q1")
    q2 = _t([128, 32], "q2")
    nc.vector.tensor_tensor(pp[:], PA[:, 0:32], PA[:, 0:32], op=OP.mult)
    nc.gpsimd.tensor_tensor(q1[:], PA[:, 32:64], PA[:, 32:64], op=OP.mult)
    nc.gpsimd.tensor_tensor(q2[:], PA[:, 64:96], PA[:, 64:96], op=OP.mult)
    nc.vector.tensor_tensor(pp[:], pp[:], q1[:], op=OP.add)
    nc.vector.tensor_tensor(pp[:], pp[:], q2[:], op=OP.add)

    # ---- plane constants
    nsq = _t([128, 4], "nsq")
    nc.vector.tensor_tensor(nsq[:], NB, NB, op=OP.mult)
    snn = _t([128, 1], "snn")
    nc.vector.tensor_reduce(snn[:], nsq[:, 0:3], axis=AX.X, op=OP.add)
    inv_nn = _t([128, 1], "inv_nn")
    nc.vector.reciprocal(inv_nn[:], snn[:])
    pinv2 = _t([128, 1], "pinv2")
    nc.scalar.mul(pinv2[:], inv_nn[:], 2.0)
    c4d = _t([128, 1], "c4d")
    nc.vector.tensor_tensor(c4d[:], PA[:, 100:101], inv_nn[:], op=OP.mult)
    nc.scalar.mul(c4d[:], c4d[:], 4.0)

    # ---- s = p.n + d over all 4096 points
    s_all = _t([128, 32], "s_all")
    t1 = _t([128, 32], "t1")
    t2 = _t([128, 32], "t2")
    nc.scalar.mul(s_all[:], PA[:, 0:32], PA[:, 97:98])
    nc.vector.tensor_scalar(t1[:], PA[:, 32:64], PA[:, 98:99], None, op0=OP.mult)
    nc.gpsimd.tensor_scalar(
        t2[:], PA[:, 64:96], PA[:, 99:100], PA[:, 100:101],
        op0=OP.mult, op1=OP.add,
    )
    nc.vector.tensor_tensor(s_all[:], s_all[:], t1[:], op=OP.add)
    nc.vector.tensor_tensor(s_all[:], s_all[:], t2[:], op=OP.add)
    M1f = _t([128, 32], "M1f")
    nc.vector.tensor_scalar(M1f[:], s_all[:], 0.0, None, op0=OP.is_lt)
    pw1 = _t([128, 32], "pw1")
    nc.gpsimd.tensor_scalar(
        pw1[:], s_all[:], 0.0, PEN, op0=OP.is_lt, op1=OP.mult
    )

    # ---- c1/c2 + reciprocals (identical on every core)
    c1row = _t([128, 1], "c1row")
    nc.vector.tensor_reduce(c1row[:], M1f[:], axis=AX.X, op=OP.add)
    c1ps = pss.tile([1, 1], FP, tag="ps")
    nc.tensor.matmul(c1ps[:], c1row[:], ones_c, start=True, stop=True)
    c1 = _t([1, 1], "c1")
    nc.scalar.copy(c1[:], c1ps[:])
    c2 = _t([1, 1], "c2")
    nc.vector.tensor_scalar(c2[:], c1[:], -1.0, float(N), op0=OP.mult, op1=OP.add)
    nc.vector.tensor_scalar_max(c1[:], c1[:], 1.0)
    nc.vector.tensor_scalar_max(c2[:], c2[:], 1.0)
    rc1 = _t([1, 1], "rc1")
    nc.vector.reciprocal(rc1[:], c1[:])
    rc2 = _t([1, 1], "rc2")
    nc.vector.reciprocal(rc2[:], c2[:])

    # ---- A side: an = -(reflected p) = (2s/nn)*n - p  (negated so PSUM
    # holds -(D+pen) and mins become maxes); B side: bn = -2p
    nalpha = _t([128, 32], "nalpha")
    nc.scalar.mul(nalpha[:], s_all[:], pinv2[:])
    AN = _t([128, 96], "AN")
    tv0 = _t([128, 32], "tv0")
    tv1 = _t([128, 32], "tv1")
    tv2 = _t([128, 32], "tv2")
    nc.scalar.mul(tv0[:], nalpha[:], PA[:, 97:98])
    nc.vector.tensor_scalar(tv1[:], nalpha[:], PA[:, 98:99], None, op0=OP.mult)
    nc.gpsimd.tensor_scalar(tv2[:], nalpha[:], PA[:, 99:100], None, op0=OP.mult)
    nc.vector.tensor_tensor(AN[:, 0:32], tv0[:], PA[:, 0:32], op=OP.subtract)
    nc.gpsimd.tensor_tensor(AN[:, 32:64], tv1[:], PA[:, 32:64], op=OP.subtract)
    nc.vector.tensor_tensor(AN[:, 64:96], tv2[:], PA[:, 64:96], op=OP.subtract)

    # ---- rank-1 rows: rrA_neg = -(|a|^2 + P*(1-m1)), ccB = |p|^2 + P*m1
    t3 = _t([128, 32], "t3")
    nc.scalar.mul(t3[:], s_all[:], c4d[:])
    u_ = _t([128, 32], "u_")
    nc.vector.tensor_tensor(u_[:], pp[:], t3[:], op=OP.add)  # |a|^2
    v_ = _t([128, 32], "v_")
    nc.gpsimd.tensor_scalar(v_[:], pw1[:], 1.0, -PEN, op0=OP.mult, op1=OP.add)
    RRn = _t([128, 32], "RRn")
    nc.vector.tensor_tensor(RRn[:], v_[:], u_[:], op=OP.subtract)
    CCp = _t([128, 32], "CCp")
    nc.gpsimd.tensor_tensor(CCp[:], pp[:], pw1[:], op=OP.add)

    # ---- hi/lo bf16 splits into the two flat K-composites
    # K pairing: [aH*bH (0:3), aH*bL (3:6), aL*bH (6:9), rrh*1 (9),
    #             rrl*1 (10), -1*cch (11), -1*ccl (12)]  (K=13)
    LOA = _t([128, 96], "LOA")
    nc.scalar.copy(CALL_A[:, 0:96], AN[:])
    nc.scalar.copy(CALL_A[:, 96:192], AN[:])
    nc.vector.tensor_tensor(LOA[:], AN[:], CALL_A[:, 0:96], op=OP.subtract)
    nc.scalar.copy(CALL_A[:, 192:288], LOA[:])

    LOR = _t([128, 32], "LOR")
    nc.scalar.copy(CALL_A[:, 288:320], RRn[:])
    nc.vector.tensor_tensor(LOR[:], RRn[:], CALL_A[:, 288:320], op=OP.subtract)
    nc.scalar.copy(CALL_A[:, 320:352], LOR[:])

    LOC = _t([128, 32], "LOC")
    nc.scalar.copy(CALL_B[:, 352:384], CCp[:])
    nc.vector.tensor_tensor(LOC[:], CCp[:], CALL_B[:, 352:384], op=OP.subtract)
    nc.scalar.copy(CALL_B[:, 384:416], LOC[:])

    # ---- K-major images via DRAM round trip, chunked by i-range so the
    # loop starts as soon as the first chunks land (64B runs on write,
    # contiguous read-back)
    CH = [(0, 16), (16, 32), (32, 64), (64, 96), (96, 128)]
    stgA = [drm.tile([13, b - a, 32], BF, name=f"stgA{r}") for r, (a, b) in enumerate(CH)]
    stgB = [drm.tile([13, b - a, 32], BF, name=f"stgB{r}") for r, (a, b) in enumerate(CH)]
    TAFs = [_t([13, b - a, 32], f"TAF{r}", BF) for r, (a, b) in enumerate(CH)]
    TBFs = [_t([13, b - a, 32], f"TBF{r}", BF) for r, (a, b) in enumerate(CH)]

    def _wA(q, r):
        a, b = CH[r]
        q.dma_start(
            stgA[r][:].rearrange("k i s -> i k s"),
            CALL_A[a:b, :].rearrange("i (k s) -> i k s", k=13),
        )
    def _wB(q, r):
        a, b = CH[r]
        q.dma_start(
            stgB[r][:].rearrange("k i s -> i k s"),
            CALL_B[a:b, :].rearrange("i (k s) -> i k s", k=13),
        )
    def _rA(q, r):
        q.dma_start(TAFs[r][:], stgA[r][:])
    def _rB(q, r):
        q.dma_start(TBFs[r][:], stgB[r][:])

    sy, sc = nc.sync, nc.scalar
    _wA(sc, 0); _rA(sc, 0); _wB(sy, 0); _rB(sy, 0)
    _wA(sy, 1); _rA(sy, 1); _wB(sc, 1); _rB(sc, 1)
    _wA(sc, 2); _rA(sc, 2); _wB(sy, 2); _rB(sy, 2)
    _wA(sy, 3); _rA(sy, 3); _wB(sc, 3); _rB(sc, 3)
    _wA(sc, 4); _rA(sc, 4); _wB(sy, 4); _rB(sy, 4)

    def _sl(islot4):
        # returns (chunk index, local i-slot offset) for a 4-slot group
        for r, (a, b) in enumerate(CH):
            if islot4 * 4 < b:
                return r, islot4 * 4 - a
        raise AssertionError

    # ---- PE clock warmup: 8 throwaway matmuls on the ready composite,
    # emitted just before the loop so the PE reaches 2.4 GHz; outputs land
    # in rotated psf tiles that the loop overwrites (finite values, never
    # read).
    for _ in range(8):
        pbw = psf.tile([128, 512], FP, name="pb")
        nc.tensor.matmul(
            pbw[:, 0:384], CALL_A[0:13, 0:128], CALL_A[0:13, 0:384],
            start=True, stop=True,
        )

    # ---- main loop: 64 matmuls, interleaved block B (col dir) and block
    # A (row dir). Image tile m = points {128m..128m+127}. PSUM holds
    # -(D + penalties).
    D2A = _t([128, 4, 8], "D2A")
    for i in range(G):
        pb = psf.tile([128, 512], FP, name="pb")
        ra, oa = _sl(i)
        nc.tensor.matmul(
            pb[:], TAFs[ra][:, oa : oa + 4, :],
            TBFs[0][:, 0:16, :],
            start=True, stop=True,
        )
        FS = fsp.tile([128, 512], HF, name="FS")
        nc.scalar.copy(FS[:], pb[:])
        nc.vector.tensor_tensor(CM1[:], CM1[:], FS[:], op=OP.max)

        g, j = i % 4, i // 4
        pa_ = psf.tile([128, 512], FP, name="pa")
        rb, ob = _sl(4 * j)
        nc.tensor.matmul(
            pa_[:], TAFs[0][:, 4 * g : 4 * (g + 1), :],
            TBFs[rb][:, ob : ob + 16, :],
            start=True, stop=True,
        )
        nc.vector.tensor_reduce(D2A[:, g, j : j + 1], pa_[:], axis=AX.X, op=OP.max)

    # ---- block A finish: d2 per own row, sentinel mask, fused sum
    d2g = _t([128, 4], "d2g")
    nc.vector.tensor_reduce(d2g[:], D2A[:], axis=AX.X, op=OP.max)
    msk2 = _t([128, 4], "msk2")
    nc.vector.tensor_scalar(msk2[:], d2g[:], SENT, None, op0=OP.is_gt)
    w2 = _t([128, 4], "w2")
    nc.vector.tensor_tensor(w2[:], msk2[:], d2g[:], op=OP.mult)
    w2s = _t([128, 1], "w2s")
    nc.vector.tensor_reduce(w2s[:], w2[:], axis=AX.X, op=OP.add)
    s2ps = pss.tile([1, 1], FP, tag="ps")
    nc.tensor.matmul(s2ps[:], w2s[:], ones128[:], start=True, stop=True)
    s2 = _t([1, 1], "s2")
    nc.scalar.copy(s2[:], s2ps[:])

    # ---- block B finish: col-min across partitions, sentinel mask, sum
    pt2 = ptp.tile([128, 4, 128], HF, name="pt")
    for t in range(4):
        nc.tensor.transpose(
            pt2[:, t, :], CM1[:, 128 * t : 128 * (t + 1)], IDN[:]
        )
    d1b = _t([128, 4], "d1b")
    nc.vector.tensor_reduce(d1b[:], pt2[:], axis=AX.X, op=OP.max)
    msk1 = _t([128, 4], "msk1")
    nc.vector.tensor_scalar(msk1[:], d1b[:], SENT, None, op0=OP.is_gt)
    w1 = _t([128, 4], "w1")
    nc.vector.tensor_tensor(w1[:], msk1[:], d1b[:], op=OP.mult)
    w1s = _t([128, 1], "w1s")
    nc.vector.tensor_reduce(w1s[:], w1[:], axis=AX.X, op=OP.add)
    s1ps = pss.tile([1, 1], FP, tag="ps")
    nc.tensor.matmul(s1ps[:], w1s[:], ones128[:], start=True, stop=True)
    s1 = _t([1, 1], "s1")
    nc.scalar.copy(s1[:], s1ps[:])

    # ---- combine: out_c = -50*(s1/c2 + s2/c1) (+ 0 from warmup)
    a1 = _t([1, 1], "a1")
    nc.vector.tensor_tensor(a1[:], s1[:], rc2[:], op=OP.mult)
    a2 = _t([1, 1], "a2")
    nc.vector.tensor_tensor(a2[:], s2[:], rc1[:], op=OP.mult)
    res = _t([1, 1], "res")
    nc.vector.tensor_tensor(res[:], a1[:], a2[:], op=OP.add)
    nc.scalar.mul(res[:], res[:], -50.0)

    if USE_AR:
        pay = drm.tile([1, 1], FP, name="pay")
        pay2 = drm.tile([1, 1], FP, name="pay2")
        nc.gpsimd.dma_start(pay[:], res[:])
        nc.gpsimd.collective_compute(
            "AllReduce",
            OP.add,
            replica_groups=[list(range(NCORES))],
            ins=[pay.opt()],
            outs=[pay2.opt()],
        )
        res2 = _t([1, 1], "res2")
        nc.gpsimd.dma_start(res2[:], pay2[:])
        nc.sync.dma_start(out_ap[:], res2[:])
    else:
        nc.sync.dma_start(out_ap[:], res[:])

    for p in (psf, pss, ptp, per, fsp, drm):
        p.seal()


_NC = None


def build():
    global _NC
    if _NC is not None:
        return _NC
    nc = bacc.Bacc(
        "TRN2", target_bir_lowering=False, debug=False, num_devices=NCORES
    )
    pa_ap = nc.dram_tensor("pa", [128, 101], FP, kind="ExternalInput").ap()
    idn_ap = nc.dram_tensor("idn", [128, 128], HF, kind="ExternalInput").ap()
    out_ap = nc.dram_tensor("out", [1, 1], FP, kind="ExternalOutput").ap()
    with tile.TileContext(nc) as tc:
        _emit(tc, out_ap, pa_ap, idn_ap)
    nc.compile()
    _NC = nc
    return nc


def make_in_maps(norm, points):
    norm = np.ascontiguousarray(norm, dtype=np.float32).reshape(1, 4)
    pts = np.ascontiguousarray(points, dtype=np.float32)
    maps = []
    for c in range(NCORES):
        rolled = np.concatenate([pts[512 * c :], pts[: 512 * c]], axis=0)
        pa = rolled.reshape(128, G, 3).transpose(0, 2, 1).reshape(128, 96)
        full = np.concatenate(
            [pa, np.ones((128, 1), np.float32), np.tile(norm, (128, 1))], axis=1
        )
        maps.append(
            {
                "pa": np.ascontiguousarray(full),
                "idn": np.eye(128, dtype=np.float16),
            }
        )
    return maps


def combine_outs(outs):
    if USE_AR:
        return np.float32(outs[0])
    return np.float32(np.sum(np.asarray(outs, dtype=np.float64)))


LAST_RESULTS = None


def kernel(norm, points):
    global LAST_RESULTS
    nc = build()
    maps = make_in_maps(norm, points)
    trace = bool(os.environ.get("KERNEL_TRACE"))
    LAST_RESULTS = run_bass_kernel_spmd(
        nc, maps, list(range(NCORES)), trace=trace
    )
    outs = [
        np.asarray(r["out"], dtype=np.float32).reshape(())
        for r in LAST_RESULTS.results
    ]
    return np.asarray(combine_outs(outs), dtype=np.float32).reshape(())
